# revision 11
# baseline (speedup 1.0000x reference)
"""Bidirectional ConvLSTM encoder kernel for Trainium2 (Bass/Tile).

Problem: B=8, T=16, C=3, H=W=32, HID=64, 7x7 convs, bidirectional.
Sharding: data-parallel over batch; core b handles batch element b, running
both the forward and backward recurrences (2 independent recurrences that
ping-pong on the PE so gate/elementwise latency of one hides under the
other's matmuls).

Conv formulation: hidden 7x7 conv (64->256ch) is computed as a sum of
shifted matmuls over a zero-padded [64, 38, 38] state image. Taps are
packed in pairs onto the 128-deep contraction dim by keeping TWO copies of
the padded state: copy "A" holds (rows 0:64 = state, rows 64:128 = state
shifted down one row) pairing kernel rows (0,1),(2,3),(4,5); copy "B"
holds (rows 64:128 = state shifted right one col) pairing row-6 taps along
kw. The leftover singleton tap (6,6) is fused with the second im2col tile
of the input conv into one K=96 matmul through a per-step staging tile
(rows 0:32 = im2col rows 128:160, rows 32:96 = plain h), so each
step-direction needs 26 matmuls per output quadrant instead of 27. All
matmul operands are fp16 (gates/cell state stay fp32; PSUM accumulates
fp32); walrus rejects mixed 32/16-bit matmul inputs and the fp32r
self-loading weight path trips a sync-wait-slot limit, so fp16 everywhere
on the PE it is.

DMA scheduling: the 16 HW DMA engines serve all in-flight descriptors
concurrently at equal rates, so a monolithic input load makes the hidden
weights land ~48us in (stalling the PE ~25us). Instead the x slices are
double-buffered per timestep (WAR deps provide demand-driven flow
control) and whh is DMA'd in 4 chunks right after the t=0/t=1 slices, so
the PE's first hidden matmuls start as soon as the t=0 elementwise chain
finishes. Output is written as fp16 (host casts to f32); the hidden state
already round-trips through fp16 so this costs nothing numerically.
"""

import numpy as np

HID = 64
T = 16
CIN = 3
H = 32
W = 32
HWSZ = H * W
PW = 38  # padded image width (32 + 2*3)
PAD = 3
KS = 7
NCORES = 8
KIN = CIN * KS * KS  # 147
KIN_PAD = 160  # 128 + 32 (zero-padded tail so the 2nd K-tile is a clean 32)

# Hidden-conv tap pairs: (kind, kh, kw).
#  "A": taps (kh, kw) + (kh+1, kw) via the row-shifted upper copy.
#  "B": taps (kh, kw) + (kh, kw+1) via the col-shifted upper copy.
# The singleton CENTER tap (3, 3) lives in the wS fold (see pack_ws): its
# rhs window is plain unshifted h, which is exactly what gets staged into
# ST[32:96] each step. The other 48 taps form a domino tiling of the 7x7
# grid minus its center.
PAIRS = (
    [("A", 0, c) for c in range(KS)]
    + [("A", 5, c) for c in range(KS)]
    + [("A", 2, 6), ("A", 3, 2)]
    + [("B", 2, 0), ("B", 2, 2), ("B", 2, 4)]
    + [("B", 3, 0), ("B", 3, 4)]
    + [("B", 4, 0), ("B", 4, 3), ("B", 4, 5)]
)
NPAIR = len(PAIRS)  # 24


def pack_whh(w_hh_f: np.ndarray, w_hh_b: np.ndarray) -> np.ndarray:
    """Pack hidden weights into lhsT tiles: [128(k), 2(dir), 24(pair), 2(mg), 128(m)]."""
    out = np.zeros((2, NPAIR, 2, 128, 128), np.float32)  # d, p, mg, k, m
    for d, wsrc in enumerate([w_hh_f, w_hh_b]):
        wsrc = np.asarray(wsrc, dtype=np.float32)  # [256, 64, 7, 7]
        for p, (kind, r, c) in enumerate(PAIRS):
            if kind == "A":
                lo, hi = (r, c), (r + 1, c)
            else:
                lo, hi = (r, c), (r, c + 1)
            for mg in range(2):
                wm = wsrc[mg * 128 : (mg + 1) * 128]  # [128, 64, 7, 7]
                out[d, p, mg, 0:64, :] = wm[:, :, lo[0], lo[1]].T
                out[d, p, mg, 64:128, :] = wm[:, :, hi[0], hi[1]].T
    return np.ascontiguousarray(out.transpose(3, 0, 1, 2, 4).astype(np.float16))  # [k, d, p, mg, m]


def pack_wih(w_ih_f: np.ndarray, w_ih_b: np.ndarray) -> np.ndarray:
    """Pack input weights (im2col rows 0:128): [128(k), 2(dir), 2(mg), 128(m)]."""
    out = np.zeros((128, 2, 2, 128), np.float32)
    for d, wsrc in enumerate([w_ih_f, w_ih_b]):
        wk = np.asarray(wsrc, dtype=np.float32).reshape(256, KIN)  # (cin,kh,kw) C-order
        for mg in range(2):
            out[:, d, mg, :] = wk[mg * 128 : (mg + 1) * 128, 0:128].T
    return np.ascontiguousarray(out.astype(np.float16))


def pack_ws(w_ih_f, w_hh_f, w_ih_b, w_hh_b) -> np.ndarray:
    """Fold lhsT [96(k), 2(dir), 2(mg), 128(m)]: rows 0:32 = im2col rows
    128:160 of w_ih, rows 32:96 = the (6,6) singleton tap of w_hh."""
    out = np.zeros((96, 2, 2, 128), np.float32)
    for d, (wi, wh) in enumerate([(w_ih_f, w_hh_f), (w_ih_b, w_hh_b)]):
        wk = np.asarray(wi, dtype=np.float32).reshape(256, KIN)
        wh = np.asarray(wh, dtype=np.float32)  # [256, 64, 7, 7]
        for mg in range(2):
            out[0 : KIN - 128, d, mg, :] = wk[mg * 128 : (mg + 1) * 128, 128:KIN].T
            out[32:96, d, mg, :] = wh[mg * 128 : (mg + 1) * 128, :, 3, 3].T
    return np.ascontiguousarray(out.astype(np.float16))


def pack_bias(b_ih_f, b_hh_f, b_ih_b, b_hh_b) -> np.ndarray:
    """[128(k), 2(dir), 2(mg)]: per-gate-channel bias."""
    out = np.zeros((128, 2, 2), np.float32)
    for d, (bi, bh) in enumerate([(b_ih_f, b_hh_f), (b_ih_b, b_hh_b)]):
        s = np.asarray(bi, dtype=np.float32) + np.asarray(bh, dtype=np.float32)  # [256]
        out[:, d, 0] = s[0:128]
        out[:, d, 1] = s[128:256]
    return np.ascontiguousarray(out)


def pack_xcol(xb: np.ndarray) -> np.ndarray:
    """im2col one batch element [T,3,32,32] -> [160(k), T, 2, 512]."""
    xb = np.asarray(xb, dtype=np.float32)
    xpad = np.pad(xb, ((0, 0), (0, 0), (PAD, PAD), (PAD, PAD)))
    win = np.lib.stride_tricks.sliding_window_view(xpad, (KS, KS), axis=(2, 3))
    # win: [T, 3, 32, 32, 7, 7] -> [(cin, kh, kw), T, hw]
    xcol = win.transpose(1, 4, 5, 0, 2, 3).reshape(KIN, T, HWSZ)
    out = np.zeros((KIN_PAD, T, 2, 512), np.float16)
    out[:KIN] = xcol.reshape(KIN, T, 2, 512).astype(np.float16)
    return out


def build_nc():
    import concourse.mybir as mybir
    from concourse import bacc
    from concourse.tile import TileContext

    F32 = mybir.dt.float32
    F16 = mybir.dt.float16
    AF = mybir.ActivationFunctionType

    nc = bacc.Bacc()
    xcol_d = nc.declare_dram_parameter("xcol", [KIN_PAD, T, 2, 512], F16, isOutput=False)
    whh_d = nc.declare_dram_parameter("whh", [128, 2, NPAIR, 2, 128], F16, isOutput=False)
    wih_d = nc.declare_dram_parameter("wih", [128, 2, 2, 128], F16, isOutput=False)
    ws_d = nc.declare_dram_parameter("ws", [96, 2, 2, 128], F16, isOutput=False)
    bias_d = nc.declare_dram_parameter("bias", [128, 2, 2], F32, isOutput=False)
    out_d = nc.declare_dram_parameter("out", [T, 2, HID, 2, 512], F16, isOutput=True)

    def tsrc_of(t, d):
        return t if d == 0 else T - 1 - t

    with TileContext(nc) as tc:
        with (
            tc.tile_pool(name="wpool", bufs=1) as wpool,
            tc.tile_pool(name="state", bufs=1) as spool,
            tc.tile_pool(name="xin", bufs=1) as xpool,
            tc.tile_pool(name="work", bufs=1) as wkpool,
            tc.tile_pool(name="psum", bufs=1, space="PSUM") as pspool,
        ):
            bias = wpool.tile([128, 2, 2], F32)
            nc.sync.dma_start(bias[:], bias_d[:])
            wih0 = wpool.tile([128, 2, 2, 128], F16)
            nc.sync.dma_start(wih0[:], wih_d[:])
            wS = wpool.tile([96, 2, 2, 128], F16)
            nc.sync.dma_start(wS[:], ws_d[:])

            # x slices, double-buffered on t-parity per direction.
            xa = xpool.tile([128, 2, 2, 2, 512], F16)  # [k, dir, parity, nh, n]
            ST = [spool.tile([96, 2, 512], F16, tag=f"st{d}", name=f"st{d}") for d in range(2)]
            for d in range(2):
                nc.sync.dma_start(xa[:, d, 0], xcol_d[0:128, tsrc_of(0, d)])
                nc.sync.dma_start(ST[d][0:32], xcol_d[128:KIN_PAD, tsrc_of(0, d)])

            # The 16 HW DMA engines fair-share bandwidth across ALL in-flight
            # descriptors, so the bulk loads are staged behind 2-byte "gate"
            # DMAs (WAW overlap with each bulk region) to keep the tiny t=0
            # slices from being starved: layer 1 (whh dir-0 + the t=1 x
            # slices) waits for xa t=0; layer 2 (whh dir-1, not needed until
            # ~35us) waits for whh dir-0. The gate garbage bytes are
            # overwritten by the real transfer.
            whh = wpool.tile([128, 2, NPAIR, 2, 128], F16)
            g0 = xa[0:1, 0:1, 0:1, 0:1, 0:1]
            nc.sync.dma_start(whh[0:1, 0, 0:1, 0:1, 0:1], g0)
            nc.sync.dma_start(whh[0:1, 0, 12:13, 0:1, 0:1], g0)
            nc.sync.dma_start(whh[:, 0, 0:12], whh_d[:, 0, 0:12])
            nc.sync.dma_start(whh[:, 0, 12:24], whh_d[:, 0, 12:24])
            for d in range(2):
                nc.sync.dma_start(xa[0:1, d, 1:2, 0:1, 0:1], g0)
                nc.sync.dma_start(xa[:, d, 1], xcol_d[0:128, tsrc_of(1, d)])
            g1 = whh[0:1, 0, 23:24, 0:1, 0:1]
            nc.sync.dma_start(whh[0:1, 1, 0:1, 0:1, 0:1], g1)
            nc.sync.dma_start(whh[0:1, 1, 12:13, 0:1, 0:1], g1)
            nc.sync.dma_start(whh[:, 1, 0:12], whh_d[:, 1, 0:12])
            nc.sync.dma_start(whh[:, 1, 12:24], whh_d[:, 1, 12:24])

            hAB = [spool.tile([128, 2, PW, PW], F16, tag=f"hAB{d}", name=f"hAB{d}") for d in range(2)]
            # cell state lives on partitions 64-127, where the f and o gates land
            c2 = [spool.tile([128, 2, 512], F32, tag=f"c{d}", name=f"c{d}") for d in range(2)]
            for tl in hAB:
                nc.vector.memset(tl[:], 0.0)

            for t in range(T):
                for d in range(2):
                    tsrc = tsrc_of(t, d)

                    ps0 = pspool.tile([128, 2, 512], F32, tag=f"ps{d}0")
                    ps1 = pspool.tile([128, 2, 512], F32, tag=f"ps{d}1")
                    pst = [ps0, ps1]

                    for mg in range(2):
                        # taps: list of (lhsT, rhs_fn(nh))
                        taps = [
                            (wih0[:, d, mg], lambda nh: xa[:, d, t % 2, nh]),
                        ]
                        if t > 0:
                            # K=96 fold: im2col tail rows + the (6,6) h tap
                            taps.append((wS[:, d, mg], lambda nh: ST[d][:, nh]))
                            for p, (kind, r, c) in enumerate(PAIRS):
                                if kind == "A":
                                    taps.append((
                                        whh[:, d, p, mg],
                                        lambda nh, r=r, c=c: hAB[d][:, 0, r + 16 * nh : r + 16 * nh + 16, c : c + 32],
                                    ))
                                else:
                                    taps.append((
                                        whh[:, d, p, mg],
                                        lambda nh, r=r, c=c: hAB[d][:, 1, r + 16 * nh : r + 16 * nh + 16, c : c + 32],
                                    ))
                        else:
                            taps.append((wS[0:32, d, mg], lambda nh: ST[d][0:32, nh]))
                        n = len(taps)
                        for i, (lh, rhf) in enumerate(taps):
                            for nh in range(2):
                                nc.tensor.matmul(
                                    pst[mg][:, nh],
                                    lh,
                                    rhf(nh),
                                    start=(i == 0),
                                    stop=(i == n - 1),
                                )

                    sif = wkpool.tile([128, 2, 512], F32, tag=f"sif{d}")
                    sgo = wkpool.tile([128, 2, 512], F32, tag=f"sgo{d}")
                    tmp = wkpool.tile([HID, 2, 512], F32, tag=f"tmp{d}")
                    tup = wkpool.tile([128, 2, 512], F32, tag=f"tup{d}")
                    th = wkpool.tile([128, 2, 512], F32, tag=f"th{d}")
                    h16 = wkpool.tile([128, 2, 512], F16, tag=f"h16{d}")
                    hl = wkpool.tile([HID, HWSZ], F16, tag=f"hl{d}")

                    # nh-split the gate chain at the pipeline edges (t=0 feeds
                    # the first hidden matmuls, t=T-1 is the exposed tail);
                    # steady-state latency hides under ~43us of matmuls.
                    halves = (0, 1) if t in (0, T - 1) else (slice(0, 2),)
                    for q in halves:
                        qq = slice(q, q + 1) if isinstance(q, int) else q
                        # gates: i,f = sigmoid(mg0); g = tanh(mg1 lo); o = sigmoid(mg1 hi)
                        nc.scalar.activation(sif[:, qq], ps0[:, qq], AF.Sigmoid, bias=bias[:, d, 0:1])
                        nc.scalar.activation(sgo[0:64, qq], ps1[0:64, qq], AF.Tanh, bias=bias[0:64, d, 1:2])
                        nc.scalar.activation(sgo[64:128, qq], ps1[64:128, qq], AF.Sigmoid, bias=bias[64:128, d, 1:2])
                        # i*g on partitions 0-63, then ship it up to 64-127 where f/o live
                        nc.vector.tensor_mul(tmp[:, qq], sif[0:64, qq], sgo[0:64, qq])
                        nc.scalar.dma_start(tup[64:128, qq], tmp[:, qq])
                        if t > 0:
                            nc.vector.tensor_mul(c2[d][64:128, qq], c2[d][64:128, qq], sif[64:128, qq])
                            nc.vector.tensor_add(c2[d][64:128, qq], c2[d][64:128, qq], tup[64:128, qq])
                        else:
                            nc.vector.tensor_copy(c2[d][64:128, qq], tup[64:128, qq])
                        src = tup if t == 0 else c2[d]
                        nc.scalar.activation(th[64:128, qq], src[64:128, qq], AF.Tanh)
                        # h = o * tanh(c), entirely on partitions 64-127, fp16 out
                        nc.vector.tensor_mul(h16[64:128, qq], sgo[64:128, qq], th[64:128, qq])
                        nc.scalar.dma_start(out_d[tsrc, d][:, qq], h16[64:128, qq])
                    if t < T - 1:
                        o3 = sgo[64:128].rearrange("p a b -> p (a b)").rearrange("p (a b) -> p a b", a=H)
                        th3 = th[64:128].rearrange("p a b -> p (a b)").rearrange("p (a b) -> p a b", a=H)
                        # shifted upper state copies written directly by lane-aligned DVE
                        nc.vector.tensor_mul(hAB[d][64:128, 0, 2:34, 3:35], o3, th3)
                        nc.vector.tensor_mul(hAB[d][64:128, 1, 3:35, 2:34], o3, th3)
                        # lower copies: ship h down to partitions 0-63, broadcast-write both
                        nc.scalar.dma_start(hl[:], h16[64:128])
                        hl4 = hl[:].rearrange("p (a b) -> p a b", a=H).unsqueeze(1).to_broadcast([HID, 2, H, W])
                        nc.vector.tensor_copy(hAB[d][0:64, :, 3:35, 3:35], hl4)
                        # stage next step's K=96 fold rhs + x parity slice
                        nc.sync.dma_start(ST[d][32:96], h16[64:128])
                        nc.sync.dma_start(ST[d][0:32], xcol_d[128:KIN_PAD, tsrc_of(t + 1, d)])
                        if t + 2 < T:
                            nc.sync.dma_start(xa[:, d, t % 2], xcol_d[0:128, tsrc_of(t + 2, d)])
    nc.compile()
    return nc


_CACHE = {}


def get_nc():
    if "nc" not in _CACHE:
        _CACHE["nc"] = build_nc()
    return _CACHE["nc"]


def make_in_maps(inputs):
    shared = {
        "whh": pack_whh(inputs["w_hh_f"], inputs["w_hh_b"]),
        "wih": pack_wih(inputs["w_ih_f"], inputs["w_ih_b"]),
        "ws": pack_ws(
            inputs["w_ih_f"], inputs["w_hh_f"], inputs["w_ih_b"], inputs["w_hh_b"]
        ),
        "bias": pack_bias(
            inputs["b_ih_f"], inputs["b_hh_f"], inputs["b_ih_b"], inputs["b_hh_b"]
        ),
    }
    x = np.asarray(inputs["x"], dtype=np.float32)
    return [dict(shared, xcol=pack_xcol(x[b])) for b in range(NCORES)]


def assemble(results):
    final = np.empty((NCORES, T, 2 * HID, H, W), np.float32)
    for b in range(NCORES):
        ob = np.asarray(results[b]["out"], dtype=np.float32).reshape(T, 2, HID, H, W)
        final[b, :, 0:HID] = ob[:, 0]
        final[b, :, HID:] = ob[:, 1]
    return final


def run_on_device(inputs, **kwargs):
    from concourse.bass_utils import run_bass_kernel_spmd

    nc = get_nc()
    in_maps = make_in_maps(inputs)
    res = run_bass_kernel_spmd(nc, in_maps, core_ids=list(range(NCORES)), **kwargs)
    return assemble(res.results), res


def kernel(**inputs):
    out, _ = run_on_device(inputs)
    return out


# revision 14
# speedup vs baseline: 1.0034x; 1.0034x over previous
"""Bidirectional ConvLSTM encoder kernel for Trainium2 (Bass/Tile).

Problem: B=8, T=16, C=3, H=W=32, HID=64, 7x7 convs, bidirectional.
Sharding: data-parallel over batch; core b handles batch element b, running
both the forward and backward recurrences (2 independent recurrences that
ping-pong on the PE so gate/elementwise latency of one hides under the
other's matmuls).

Conv formulation: hidden 7x7 conv (64->256ch) is computed as a sum of
shifted matmuls over a zero-padded [64, 38, 38] state image. Taps are
packed in pairs onto the 128-deep contraction dim by keeping TWO copies of
the padded state: copy "A" holds (rows 0:64 = state, rows 64:128 = state
shifted down one row) pairing kernel rows (0,1),(2,3),(4,5); copy "B"
holds (rows 64:128 = state shifted right one col) pairing row-6 taps along
kw. The leftover singleton tap (6,6) is fused with the second im2col tile
of the input conv into one K=96 matmul through a per-step staging tile
(rows 0:32 = im2col rows 128:160, rows 32:96 = plain h), so each
step-direction needs 26 matmuls per output quadrant instead of 27. All
matmul operands are fp16 (gates/cell state stay fp32; PSUM accumulates
fp32); walrus rejects mixed 32/16-bit matmul inputs and the fp32r
self-loading weight path trips a sync-wait-slot limit, so fp16 everywhere
on the PE it is.

DMA scheduling: the 16 HW DMA engines serve all in-flight descriptors
concurrently at equal rates, so a monolithic input load makes the hidden
weights land ~48us in (stalling the PE ~25us). Instead the x slices are
double-buffered per timestep (WAR deps provide demand-driven flow
control) and whh is DMA'd in 4 chunks right after the t=0/t=1 slices, so
the PE's first hidden matmuls start as soon as the t=0 elementwise chain
finishes. Output is written as fp16 (host casts to f32); the hidden state
already round-trips through fp16 so this costs nothing numerically.
"""

import numpy as np

HID = 64
T = 16
CIN = 3
H = 32
W = 32
HWSZ = H * W
PW = 38  # padded image width (32 + 2*3)
PAD = 3
KS = 7
NCORES = 8
KIN = CIN * KS * KS  # 147
KIN_PAD = 160  # 128 + 32 (zero-padded tail so the 2nd K-tile is a clean 32)

# Hidden-conv tap pairs: (kind, kh, kw).
#  "A": taps (kh, kw) + (kh+1, kw) via the row-shifted upper copy.
#  "B": taps (kh, kw) + (kh, kw+1) via the col-shifted upper copy.
# The singleton CENTER tap (3, 3) lives in the wS fold (see pack_ws): its
# rhs window is plain unshifted h, which is exactly what gets staged into
# ST[32:96] each step. The other 48 taps form a domino tiling of the 7x7
# grid minus its center.
PAIRS = (
    [("A", 0, c) for c in range(KS)]
    + [("A", 5, c) for c in range(KS)]
    + [("A", 2, 6), ("A", 3, 2)]
    + [("B", 2, 0), ("B", 2, 2), ("B", 2, 4)]
    + [("B", 3, 0), ("B", 3, 4)]
    + [("B", 4, 0), ("B", 4, 3), ("B", 4, 5)]
)
NPAIR = len(PAIRS)  # 24


def pack_whh(w_hh_f: np.ndarray, w_hh_b: np.ndarray) -> np.ndarray:
    """Pack hidden weights into lhsT tiles: [128(k), 2(dir), 24(pair), 2(mg), 128(m)]."""
    out = np.zeros((2, NPAIR, 2, 128, 128), np.float32)  # d, p, mg, k, m
    for d, wsrc in enumerate([w_hh_f, w_hh_b]):
        wsrc = np.asarray(wsrc, dtype=np.float32)  # [256, 64, 7, 7]
        for p, (kind, r, c) in enumerate(PAIRS):
            if kind == "A":
                lo, hi = (r, c), (r + 1, c)
            else:
                lo, hi = (r, c), (r, c + 1)
            for mg in range(2):
                wm = wsrc[mg * 128 : (mg + 1) * 128]  # [128, 64, 7, 7]
                out[d, p, mg, 0:64, :] = wm[:, :, lo[0], lo[1]].T
                out[d, p, mg, 64:128, :] = wm[:, :, hi[0], hi[1]].T
    return np.ascontiguousarray(out.transpose(3, 0, 1, 2, 4).astype(np.float16))  # [k, d, p, mg, m]


def pack_wih(w_ih_f: np.ndarray, w_ih_b: np.ndarray) -> np.ndarray:
    """Pack input weights (im2col rows 0:128): [128(k), 2(dir), 2(mg), 128(m)]."""
    out = np.zeros((128, 2, 2, 128), np.float32)
    for d, wsrc in enumerate([w_ih_f, w_ih_b]):
        wk = np.asarray(wsrc, dtype=np.float32).reshape(256, KIN)  # (cin,kh,kw) C-order
        for mg in range(2):
            out[:, d, mg, :] = wk[mg * 128 : (mg + 1) * 128, 0:128].T
    return np.ascontiguousarray(out.astype(np.float16))


def pack_ws(w_ih_f, w_hh_f, w_ih_b, w_hh_b) -> np.ndarray:
    """Fold lhsT [96(k), 2(dir), 2(mg), 128(m)]: rows 0:32 = im2col rows
    128:160 of w_ih, rows 32:96 = the (6,6) singleton tap of w_hh."""
    out = np.zeros((96, 2, 2, 128), np.float32)
    for d, (wi, wh) in enumerate([(w_ih_f, w_hh_f), (w_ih_b, w_hh_b)]):
        wk = np.asarray(wi, dtype=np.float32).reshape(256, KIN)
        wh = np.asarray(wh, dtype=np.float32)  # [256, 64, 7, 7]
        for mg in range(2):
            out[0 : KIN - 128, d, mg, :] = wk[mg * 128 : (mg + 1) * 128, 128:KIN].T
            out[32:96, d, mg, :] = wh[mg * 128 : (mg + 1) * 128, :, 3, 3].T
    return np.ascontiguousarray(out.astype(np.float16))


def pack_bias(b_ih_f, b_hh_f, b_ih_b, b_hh_b) -> np.ndarray:
    """[128(k), 2(dir), 2(mg)]: per-gate-channel bias."""
    out = np.zeros((128, 2, 2), np.float32)
    for d, (bi, bh) in enumerate([(b_ih_f, b_hh_f), (b_ih_b, b_hh_b)]):
        s = np.asarray(bi, dtype=np.float32) + np.asarray(bh, dtype=np.float32)  # [256]
        out[:, d, 0] = s[0:128]
        out[:, d, 1] = s[128:256]
    return np.ascontiguousarray(out)


def pack_xcol(xb: np.ndarray) -> np.ndarray:
    """im2col one batch element [T,3,32,32] -> [160(k), T, 2, 512]."""
    xb = np.asarray(xb, dtype=np.float32)
    xpad = np.pad(xb, ((0, 0), (0, 0), (PAD, PAD), (PAD, PAD)))
    win = np.lib.stride_tricks.sliding_window_view(xpad, (KS, KS), axis=(2, 3))
    # win: [T, 3, 32, 32, 7, 7] -> [(cin, kh, kw), T, hw]
    xcol = win.transpose(1, 4, 5, 0, 2, 3).reshape(KIN, T, HWSZ)
    out = np.zeros((KIN_PAD, T, 2, 512), np.float16)
    out[:KIN] = xcol.reshape(KIN, T, 2, 512).astype(np.float16)
    return out


def build_nc():
    import concourse.mybir as mybir
    from concourse import bacc
    from concourse.tile import TileContext

    F32 = mybir.dt.float32
    F16 = mybir.dt.float16
    AF = mybir.ActivationFunctionType

    nc = bacc.Bacc()
    xcol_d = nc.declare_dram_parameter("xcol", [KIN_PAD, T, 2, 512], F16, isOutput=False)
    whh_d = nc.declare_dram_parameter("whh", [128, 2, NPAIR, 2, 128], F16, isOutput=False)
    wih_d = nc.declare_dram_parameter("wih", [128, 2, 2, 128], F16, isOutput=False)
    ws_d = nc.declare_dram_parameter("ws", [96, 2, 2, 128], F16, isOutput=False)
    bias_d = nc.declare_dram_parameter("bias", [128, 2, 2], F32, isOutput=False)
    out_d = nc.declare_dram_parameter("out", [T, 2, HID, 2, 512], F16, isOutput=True)

    def tsrc_of(t, d):
        return t if d == 0 else T - 1 - t

    with TileContext(nc) as tc:
        with (
            tc.tile_pool(name="wpool", bufs=1) as wpool,
            tc.tile_pool(name="state", bufs=1) as spool,
            tc.tile_pool(name="xin", bufs=1) as xpool,
            tc.tile_pool(name="work", bufs=1) as wkpool,
            tc.tile_pool(name="psum", bufs=1, space="PSUM") as pspool,
        ):
            bias = wpool.tile([128, 2, 2], F32)
            nc.sync.dma_start(bias[:], bias_d[:])
            wih0 = wpool.tile([128, 2, 2, 128], F16)
            nc.sync.dma_start(wih0[:], wih_d[:])
            wS = wpool.tile([96, 2, 2, 128], F16)
            nc.sync.dma_start(wS[:], ws_d[:])

            # x slices, double-buffered on t-parity per direction.
            xa = xpool.tile([128, 2, 2, 2, 512], F16)  # [k, dir, parity, nh, n]
            ST = [spool.tile([96, 2, 512], F16, tag=f"st{d}", name=f"st{d}") for d in range(2)]
            for d in range(2):
                nc.sync.dma_start(xa[:, d, 0], xcol_d[0:128, tsrc_of(0, d)])
                nc.sync.dma_start(ST[d][0:32], xcol_d[128:KIN_PAD, tsrc_of(0, d)])

            whh = wpool.tile([128, 2, NPAIR, 2, 128], F16)
            for d in range(2):
                nc.sync.dma_start(whh[:, d, 0:12], whh_d[:, d, 0:12])
                nc.sync.dma_start(whh[:, d, 12:24], whh_d[:, d, 12:24])
            for d in range(2):
                nc.sync.dma_start(xa[:, d, 1], xcol_d[0:128, tsrc_of(1, d)])

            hAB = [spool.tile([128, 2, PW, PW], F16, tag=f"hAB{d}", name=f"hAB{d}") for d in range(2)]
            # cell state lives on partitions 64-127, where the f and o gates land
            c2 = [spool.tile([128, 2, 512], F32, tag=f"c{d}", name=f"c{d}") for d in range(2)]
            for tl in hAB:
                nc.vector.memset(tl[:], 0.0)

            for t in range(T):
                for d in range(2):
                    tsrc = tsrc_of(t, d)

                    ps0 = pspool.tile([128, 2, 512], F32, tag=f"ps{d}0")
                    ps1 = pspool.tile([128, 2, 512], F32, tag=f"ps{d}1")
                    pst = [ps0, ps1]

                    for mg in range(2):
                        # taps: list of (lhsT, rhs_fn(nh))
                        taps = [
                            (wih0[:, d, mg], lambda nh: xa[:, d, t % 2, nh]),
                        ]
                        if t > 0:
                            # K=96 fold: im2col tail rows + the (6,6) h tap
                            taps.append((wS[:, d, mg], lambda nh: ST[d][:, nh]))
                            for p, (kind, r, c) in enumerate(PAIRS):
                                if kind == "A":
                                    taps.append((
                                        whh[:, d, p, mg],
                                        lambda nh, r=r, c=c: hAB[d][:, 0, r + 16 * nh : r + 16 * nh + 16, c : c + 32],
                                    ))
                                else:
                                    taps.append((
                                        whh[:, d, p, mg],
                                        lambda nh, r=r, c=c: hAB[d][:, 1, r + 16 * nh : r + 16 * nh + 16, c : c + 32],
                                    ))
                        else:
                            taps.append((wS[0:32, d, mg], lambda nh: ST[d][0:32, nh]))
                        n = len(taps)
                        for i, (lh, rhf) in enumerate(taps):
                            for nh in range(2):
                                nc.tensor.matmul(
                                    pst[mg][:, nh],
                                    lh,
                                    rhf(nh),
                                    start=(i == 0),
                                    stop=(i == n - 1),
                                )

                    sif = wkpool.tile([128, 2, 512], F32, tag=f"sif{d}")
                    sgo = wkpool.tile([128, 2, 512], F32, tag=f"sgo{d}")
                    tmp = wkpool.tile([HID, 2, 512], F32, tag=f"tmp{d}")
                    tup = wkpool.tile([128, 2, 512], F32, tag=f"tup{d}")
                    th = wkpool.tile([128, 2, 512], F32, tag=f"th{d}")
                    h16 = wkpool.tile([128, 2, 512], F16, tag=f"h16{d}")
                    hl = wkpool.tile([HID, HWSZ], F16, tag=f"hl{d}")

                    # gates: i,f = sigmoid(mg0); g = tanh(mg1 lo); o = sigmoid(mg1 hi)
                    nc.scalar.activation(sif[:], ps0[:], AF.Sigmoid, bias=bias[:, d, 0:1])
                    nc.scalar.activation(sgo[0:64], ps1[0:64], AF.Tanh, bias=bias[0:64, d, 1:2])
                    nc.scalar.activation(sgo[64:128], ps1[64:128], AF.Sigmoid, bias=bias[64:128, d, 1:2])
                    # i*g on partitions 0-63, then ship it up to 64-127 where f/o live
                    nc.vector.tensor_mul(tmp[:], sif[0:64], sgo[0:64])
                    nc.sync.dma_start(tup[64:128], tmp[:])
                    if t > 0:
                        nc.vector.tensor_mul(c2[d][64:128], c2[d][64:128], sif[64:128])
                        nc.vector.tensor_add(c2[d][64:128], c2[d][64:128], tup[64:128])
                    else:
                        nc.vector.tensor_copy(c2[d][64:128], tup[64:128])
                    src = tup if t == 0 else c2[d]
                    nc.scalar.activation(th[64:128], src[64:128], AF.Tanh)
                    # h = o * tanh(c), entirely on partitions 64-127, fp16 out
                    nc.vector.tensor_mul(h16[64:128], sgo[64:128], th[64:128])
                    nc.scalar.dma_start(out_d[tsrc, d], h16[64:128])
                    if t < T - 1:
                        o3 = sgo[64:128].rearrange("p a b -> p (a b)").rearrange("p (a b) -> p a b", a=H)
                        th3 = th[64:128].rearrange("p a b -> p (a b)").rearrange("p (a b) -> p a b", a=H)
                        # shifted upper state copies written directly by lane-aligned DVE
                        nc.vector.tensor_mul(hAB[d][64:128, 0, 2:34, 3:35], o3, th3)
                        nc.vector.tensor_mul(hAB[d][64:128, 1, 3:35, 2:34], o3, th3)
                        # lower copies: ship h down to partitions 0-63, broadcast-write both
                        nc.sync.dma_start(hl[:], h16[64:128])
                        hl4 = hl[:].rearrange("p (a b) -> p a b", a=H).unsqueeze(1).to_broadcast([HID, 2, H, W])
                        nc.vector.tensor_copy(hAB[d][0:64, :, 3:35, 3:35], hl4)
                        # stage next step's K=96 fold rhs + x parity slice
                        nc.sync.dma_start(ST[d][32:96], h16[64:128])
                        nc.sync.dma_start(ST[d][0:32], xcol_d[128:KIN_PAD, tsrc_of(t + 1, d)])
                        if t + 2 < T:
                            nc.sync.dma_start(xa[:, d, t % 2], xcol_d[0:128, tsrc_of(t + 2, d)])
    nc.compile()
    return nc


_CACHE = {}


def get_nc():
    if "nc" not in _CACHE:
        _CACHE["nc"] = build_nc()
    return _CACHE["nc"]


def make_in_maps(inputs):
    shared = {
        "whh": pack_whh(inputs["w_hh_f"], inputs["w_hh_b"]),
        "wih": pack_wih(inputs["w_ih_f"], inputs["w_ih_b"]),
        "ws": pack_ws(
            inputs["w_ih_f"], inputs["w_hh_f"], inputs["w_ih_b"], inputs["w_hh_b"]
        ),
        "bias": pack_bias(
            inputs["b_ih_f"], inputs["b_hh_f"], inputs["b_ih_b"], inputs["b_hh_b"]
        ),
    }
    x = np.asarray(inputs["x"], dtype=np.float32)
    return [dict(shared, xcol=pack_xcol(x[b])) for b in range(NCORES)]


def assemble(results):
    final = np.empty((NCORES, T, 2 * HID, H, W), np.float32)
    for b in range(NCORES):
        ob = np.asarray(results[b]["out"], dtype=np.float32).reshape(T, 2, HID, H, W)
        final[b, :, 0:HID] = ob[:, 0]
        final[b, :, HID:] = ob[:, 1]
    return final


def run_on_device(inputs, **kwargs):
    from concourse.bass_utils import run_bass_kernel_spmd

    nc = get_nc()
    in_maps = make_in_maps(inputs)
    res = run_bass_kernel_spmd(nc, in_maps, core_ids=list(range(NCORES)), **kwargs)
    return assemble(res.results), res


def kernel(**inputs):
    out, _ = run_on_device(inputs)
    return out
